# revision 1
# baseline (speedup 1.0000x reference)
"""Trainium2 Bass kernel for BertSelfAttention (B=1, S=4096, HID=768, 12 heads).

Sharding: 8 cores = 4 head-groups x 2 query-halves. Each core computes 3 heads
for 2048 query rows against all 4096 keys, fused (scores never hit HBM).

Per-core dataflow (bf16 matmuls, fp32 PSUM accumulation):
  - hidden_states cast to bf16 by gpsimd DMA, transposed on the PE into
    hsT [hid, s] (tile-major layout).
  - Q^T per head duplicated on both partition halves; K^T per head stored with
    even/odd key-chunks on partition halves -> score matmuls (contraction=64)
    run as row-tiled concurrent pairs at full PE rate.
  - scores computed transposed: S^T[k, q] tiles land in PSUM; one ScalarE Exp
    instruction per [128, 1024] tile writes bf16 P^T straight to SBUF
    (scale=1/8 folded into the activation).
  - additive attention mask handled exactly by scaling V rows (and the
    appended ones-column) with exp(mask[k]).
  - V is augmented with a ones column per head, so the context matmul
    accumulates both sum(p*v) and sum(p) (the softmax denominator) in one
    PSUM group.
  - ctx^T [65, 512] tiles are PE-transposed back to [q, d] layout, divided by
    the denominator on VectorE, and DMA'd out.
"""

import sys

sys.path.insert(0, "/opt/trn_rl_repo")

import numpy as np

import concourse.bacc as bacc
import concourse.mybir as mybir
import concourse.tile as tile
from concourse import bass_utils

B, S, HID = 1, 4096, 768
NH, HD = 12, 64
N_CORES = 8
HG = 4  # head-groups (tensor parallel)
QS = 2  # query splits (data parallel on sequence)
HPC = NH // HG  # 3 heads per core
SQ = S // QS  # 2048 query rows per core
CC = HPC * HD  # 192 projection columns per core
VC = HPC * (HD + 1)  # 195 augmented V columns (ones col per head)
NHC = HID // 128  # 6 contraction chunks
NT = S // 128  # 32 key tiles
NTQ = SQ // 128  # 16 query tiles

f32 = mybir.dt.float32
bf16 = mybir.dt.bfloat16

_CACHE = {}


def _build():
    EXP = mybir.ActivationFunctionType.Exp
    nc = bacc.Bacc("TRN2", target_bir_lowering=False)

    hs_d = nc.dram_tensor("hs", [S, HID], f32, kind="ExternalInput")
    hsq_d = nc.dram_tensor("hsq", [SQ, HID], f32, kind="ExternalInput")
    wq_d = nc.dram_tensor("wq", [HID, CC], f32, kind="ExternalInput")
    wk_d = nc.dram_tensor("wk", [HID, CC], f32, kind="ExternalInput")
    wv_d = nc.dram_tensor("wv", [HID, VC], f32, kind="ExternalInput")
    bq_d = nc.dram_tensor("bq", [CC, 1], f32, kind="ExternalInput")
    bk_d = nc.dram_tensor("bk", [CC, 1], f32, kind="ExternalInput")
    bv_d = nc.dram_tensor("bv", [1, VC], f32, kind="ExternalInput")
    mask_d = nc.dram_tensor("mask", [S, 1], f32, kind="ExternalInput")
    ident_d = nc.dram_tensor("ident", [128, 128], f32, kind="ExternalInput")
    out_d = nc.dram_tensor("out", [SQ, CC], f32, kind="ExternalOutput")

    with tile.TileContext(nc) as tc:
        with (
            tc.tile_pool(name="persist", bufs=1) as P,
            tc.tile_pool(name="stage", bufs=4) as ST,
            tc.tile_pool(name="work", bufs=3) as WK,
            tc.tile_pool(name="outp", bufs=2) as OP,
            tc.tile_pool(name="ppsum", bufs=2, space="PSUM") as PP,
            tc.tile_pool(name="bpsum", bufs=2, space="PSUM") as BP,
            tc.tile_pool(name="cpsum", bufs=2, space="PSUM") as CP,
        ):
            # ---- persistent SBUF tensors ----
            hsT = P.tile([128, NT * HID], bf16, tag="hsT")  # tile-major [hid, s]
            hsTq = P.tile([128, NTQ * HID], bf16, tag="hsTq")
            wqb = P.tile([128, NHC * CC], bf16, tag="wqb")
            wkb = P.tile([128, NHC * CC], bf16, tag="wkb")
            wvb = P.tile([128, NHC * VC], bf16, tag="wvb")
            bvb = P.tile([1, VC], bf16, tag="bvb")
            bqt = P.tile([128, HPC], f32, tag="bqt")
            bkt = P.tile([128, HPC], f32, tag="bkt")
            maskt = P.tile([128, NT], f32, tag="maskt")
            wmask = P.tile([128, NT], f32, tag="wmask")
            identb = P.tile([128, 128], bf16, tag="identb")
            identf = P.tile([128, 128], f32, tag="identf")
            onesb = P.tile([1, 128], bf16, tag="onesb")
            qt = [
                P.tile([128, SQ], bf16, tag=f"qt{h}", name=f"qt{h}")
                for h in range(HPC)
            ]
            kt = [
                P.tile([128, S // 2], bf16, tag=f"kt{h}", name=f"kt{h}")
                for h in range(HPC)
            ]
            vv = P.tile([128, NT * VC], bf16, tag="vv")

            # ---- small constant loads ----
            nc.gpsimd.dma_start(identb[:], ident_d[:])  # f32 -> bf16 cast
            nc.sync.dma_start(identf[:], ident_d[:])
            nc.vector.memset(onesb[:], 1.0)
            for c in range(NHC):
                nc.gpsimd.dma_start(
                    wqb[:, c * CC : (c + 1) * CC], wq_d[c * 128 : (c + 1) * 128, :]
                )
                nc.gpsimd.dma_start(
                    wkb[:, c * CC : (c + 1) * CC], wk_d[c * 128 : (c + 1) * 128, :]
                )
                nc.gpsimd.dma_start(
                    wvb[:, c * VC : (c + 1) * VC], wv_d[c * 128 : (c + 1) * 128, :]
                )
            nc.gpsimd.dma_start(bvb[:], bv_d[:])
            for h in range(HPC):
                nc.sync.dma_start(bqt[0:64, h : h + 1], bq_d[h * 64 : (h + 1) * 64, :])
                nc.sync.dma_start(
                    bqt[64:128, h : h + 1], bq_d[h * 64 : (h + 1) * 64, :]
                )
                nc.sync.dma_start(bkt[0:64, h : h + 1], bk_d[h * 64 : (h + 1) * 64, :])
                nc.sync.dma_start(
                    bkt[64:128, h : h + 1], bk_d[h * 64 : (h + 1) * 64, :]
                )
            for t in range(NT):
                nc.sync.dma_start(maskt[:, t : t + 1], mask_d[t * 128 : (t + 1) * 128, :])
            nc.scalar.activation(wmask[:], maskt[:], EXP)

            # ---- hidden_states load + transpose (hs and hs_q) ----
            for t in range(NT):
                hsb = ST.tile([128, HID], bf16, tag="hsb", name="hsb")
                nc.gpsimd.dma_start(hsb[:], hs_d[t * 128 : (t + 1) * 128, :])
                tp = BP.tile([128, HID], bf16, tag="big", name="tpp")
                for c in range(NHC):
                    nc.tensor.transpose(
                        tp[:, c * 128 : (c + 1) * 128],
                        hsb[:, c * 128 : (c + 1) * 128],
                        identb[:],
                    )
                nc.vector.tensor_copy(hsT[:, t * HID : (t + 1) * HID], tp[:])
            for t in range(NTQ):
                hsbq = ST.tile([128, HID], bf16, tag="hsb", name="hsbq")
                nc.gpsimd.dma_start(hsbq[:], hsq_d[t * 128 : (t + 1) * 128, :])
                tpq = BP.tile([128, HID], bf16, tag="big", name="tppq")
                for c in range(NHC):
                    nc.tensor.transpose(
                        tpq[:, c * 128 : (c + 1) * 128],
                        hsbq[:, c * 128 : (c + 1) * 128],
                        identb[:],
                    )
                nc.vector.tensor_copy(hsTq[:, t * HID : (t + 1) * HID], tpq[:])

            hsT_t = hsT.rearrange("p (t x) -> p t x", x=HID)  # [128, 32, 768]
            hsT_eo = hsT.rearrange(
                "p (t two x) -> p t two x", two=2, x=HID
            )  # [128, 16, 2, 768]
            hsTq_t = hsTq.rearrange("p (t x) -> p t x", x=HID)  # [128, 16, 768]

            # ---- Q^T projection (duplicated on both partition halves) ----
            for h in range(HPC):
                for j in range(SQ // 512):
                    pq = PP.tile([128, 512], f32, tag="proj", name="pq")
                    for c in range(NHC):
                        lw = wqb[:, c * CC + h * 64 : c * CC + (h + 1) * 64]
                        rq = hsTq_t[:, 4 * j : 4 * j + 4, c * 128 : (c + 1) * 128]
                        nc.tensor.matmul(
                            pq[0:64, :], lw, rq, start=(c == 0), stop=(c == NHC - 1)
                        )
                        nc.tensor.matmul(
                            pq[64:128, :], lw, rq, start=(c == 0), stop=(c == NHC - 1)
                        )
                    nc.vector.tensor_scalar_add(
                        qt[h][:, j * 512 : (j + 1) * 512], pq[:], bqt[:, h : h + 1]
                    )

            # ---- K^T projection (even/odd key chunks on partition halves) ----
            for h in range(HPC):
                for j in range(4):
                    pk = PP.tile([128, 512], f32, tag="proj", name="pk")
                    for c in range(NHC):
                        lw = wkb[:, c * CC + h * 64 : c * CC + (h + 1) * 64]
                        re = hsT_eo[:, 4 * j : 4 * j + 4, 0, c * 128 : (c + 1) * 128]
                        ro = hsT_eo[:, 4 * j : 4 * j + 4, 1, c * 128 : (c + 1) * 128]
                        nc.tensor.matmul(
                            pk[0:64, :], lw, re, start=(c == 0), stop=(c == NHC - 1)
                        )
                        nc.tensor.matmul(
                            pk[64:128, :], lw, ro, start=(c == 0), stop=(c == NHC - 1)
                        )
                    nc.vector.tensor_scalar_add(
                        kt[h][:, j * 512 : (j + 1) * 512], pk[:], bkt[:, h : h + 1]
                    )

            # ---- V projection (augmented ones column; mask folded in) ----
            for t in range(NT):
                pv = PP.tile([128, VC], f32, tag="proj", name="pv")
                for c in range(NHC):
                    nc.tensor.matmul(
                        pv[:],
                        hsT_t[:, t, c * 128 : (c + 1) * 128],
                        wvb[:, c * VC : (c + 1) * VC],
                        start=(c == 0),
                        stop=False,
                    )
                nc.tensor.matmul(pv[:], onesb[:], bvb[:], start=False, stop=True)
                nc.vector.tensor_scalar_mul(
                    vv[:, t * VC : (t + 1) * VC], pv[:], wmask[:, t : t + 1]
                )

            # ---- attention main loop ----
            for jq in range(SQ // 512):
                for h in range(HPC):
                    cx = CP.tile([65, 512], f32, tag="ctx", name="cx")
                    for g in range(16):
                        sc = BP.tile([128, 1024], f32, tag="big", name="sc")
                        nc.tensor.matmul(
                            sc[:, 0:512],
                            kt[h][0:64, g * 128 : (g + 1) * 128],
                            qt[h][0:64, jq * 512 : (jq + 1) * 512],
                            start=True,
                            stop=True,
                        )
                        nc.tensor.matmul(
                            sc[:, 512:1024],
                            kt[h][64:128, g * 128 : (g + 1) * 128],
                            qt[h][64:128, jq * 512 : (jq + 1) * 512],
                            start=True,
                            stop=True,
                        )
                        pt = WK.tile([128, 1024], bf16, tag="pts", name="pt")
                        nc.scalar.activation(pt[:], sc[:], EXP, scale=0.125)
                        nc.tensor.matmul(
                            cx[:],
                            vv[:, (2 * g) * VC + h * 65 : (2 * g) * VC + h * 65 + 65],
                            pt[:, 0:512],
                            start=(g == 0),
                            stop=False,
                        )
                        nc.tensor.matmul(
                            cx[:],
                            vv[
                                :,
                                (2 * g + 1) * VC
                                + h * 65 : (2 * g + 1) * VC
                                + h * 65
                                + 65,
                            ],
                            pt[:, 512:1024],
                            start=False,
                            stop=(g == 15),
                        )
                    # ---- output: transpose, normalize, store ----
                    cs = OP.tile([65, 512], f32, tag="cs", name="cs")
                    nc.vector.tensor_copy(cs[:], cx[:])
                    for t4 in range(4):
                        tp2 = PP.tile([128, 65], f32, tag="proj", name="tp2")
                        nc.tensor.transpose(
                            tp2[:], cs[:, t4 * 128 : (t4 + 1) * 128], identf[0:65, 0:65]
                        )
                        rc = OP.tile([128, 1], f32, tag="rc", name="rc")
                        nc.vector.reciprocal(rc[:], tp2[:, 64:65])
                        ot = OP.tile([128, 64], f32, tag="ot", name="ot")
                        nc.vector.tensor_scalar_mul(ot[:], tp2[:, 0:64], rc[:])
                        nc.sync.dma_start(
                            out_d[
                                jq * 512 + t4 * 128 : jq * 512 + (t4 + 1) * 128,
                                h * 64 : (h + 1) * 64,
                            ],
                            ot[:],
                        )

    nc.compile()
    return nc


def _get_nc():
    if "nc" not in _CACHE:
        _CACHE["nc"] = _build()
    return _CACHE["nc"]


def _in_maps(hs, mask, Wq, bq, Wk, bk, Wv, bv):
    ident = np.eye(128, dtype=np.float32)
    maps = []
    for core in range(N_CORES):
        hg, sh = core // QS, core % QS
        csl = slice(hg * CC, (hg + 1) * CC)
        wv_aug = np.zeros((HID, VC), np.float32)
        bv_aug = np.zeros((1, VC), np.float32)
        for h in range(HPC):
            wv_aug[:, h * 65 : h * 65 + 64] = Wv[:, hg * CC + h * 64 : hg * CC + (h + 1) * 64]
            bv_aug[0, h * 65 : h * 65 + 64] = bv[hg * CC + h * 64 : hg * CC + (h + 1) * 64]
            bv_aug[0, h * 65 + 64] = 1.0
        maps.append(
            {
                "hs": hs,
                "hsq": np.ascontiguousarray(hs[sh * SQ : (sh + 1) * SQ, :]),
                "wq": np.ascontiguousarray(Wq[:, csl]),
                "wk": np.ascontiguousarray(Wk[:, csl]),
                "wv": wv_aug,
                "bq": np.ascontiguousarray(bq[csl].reshape(-1, 1)),
                "bk": np.ascontiguousarray(bk[csl].reshape(-1, 1)),
                "bv": bv_aug,
                "mask": mask,
                "ident": ident,
            }
        )
    return maps


def kernel(hidden_states, attention_mask, Wq, bq, Wk, bk, Wv, bv, **run_kwargs):
    hs = np.ascontiguousarray(np.asarray(hidden_states, np.float32).reshape(S, HID))
    mask = np.ascontiguousarray(
        np.asarray(attention_mask, np.float32).reshape(S, 1)
    )
    Wq = np.asarray(Wq, np.float32)
    Wk = np.asarray(Wk, np.float32)
    Wv = np.asarray(Wv, np.float32)
    bq = np.asarray(bq, np.float32)
    bk = np.asarray(bk, np.float32)
    bv = np.asarray(bv, np.float32)

    nc = _get_nc()
    maps = _in_maps(hs, mask, Wq, bq, Wk, bk, Wv, bv)
    res = bass_utils.run_bass_kernel_spmd(
        nc, maps, core_ids=list(range(N_CORES)), **run_kwargs
    )
    out = np.zeros((S, NH * HD), np.float32)
    for core in range(N_CORES):
        hg, sh = core // QS, core % QS
        out[sh * SQ : (sh + 1) * SQ, hg * CC : (hg + 1) * CC] = res.results[core][
            "out"
        ]
    if "trace" in run_kwargs:
        _CACHE["last_result"] = res
    return out.reshape(B, S, NH * HD)
